# revision 7
# baseline (speedup 1.0000x reference)
"""LoRA linear kernel for Trainium2 (8 NeuronCores, SPMD data-parallel).

Computes y = x @ (B @ A)^T for
    x: [4, 2048, 4096] f32, B: [4096, 16] f32, A: [16, 4096] f32.

Strategy: never materialize W = B @ A.  Factor as t = x @ A^T (rank 16)
then y = t @ B^T.  Tokens (4*2048 = 8192) are sharded across 8 cores
(1024 tokens each); A and B are replicated.

The kernel is HBM-bandwidth bound (~358 GB/s/core), so x is staged and
y is returned in float16 (tolerance is 2e-2; fp16 end-to-end gives
~7e-4), halving HBM traffic vs fp32.  Matmuls run fp16 x fp16; mm1
accumulates fp32 in PSUM, mm2 is single-shot so it drains fp16
directly to PSUM (1024 fp16 = one bank) which doubles both the mm2
free dim and the PSUM-evacuation rate (16-bit copies get the 2x
perf mode; evacuation alternates DVE/ACT so neither engine binds).

x is staged chunk-major ([chunk, 128, 16, 512]) so each x DMA reads
16 KiB contiguous per partition -> near line-rate descriptors.

Per-core dataflow:
  mm1: t^T[16, tok]  = sum_ko  A^T[ko]  (lhsT [128,16]) . x^T[ko] (rhs [128,512])
  mm2: y[tok128, o]  = t^T[:, chunk] (lhsT [16,128])    . B^T     (rhs [16,1024])
  y DMA'd out in natural token-major layout -> host just concatenates.
"""

import sys

import numpy as np

if "/opt/trn_rl_repo" not in sys.path:
    sys.path.insert(0, "/opt/trn_rl_repo")

# Problem shape (hardcoded per contract)
BATCH = 4
SEQ = 2048
D = 4096          # in_features == out_features
R = 16            # lora rank
NCORES = 8
NTOK = BATCH * SEQ            # 8192 tokens total
TOK = NTOK // NCORES          # 1024 tokens per core
P = 128                       # partitions
KO = D // P                   # 32 feature chunks
TB = 512                      # token block for mm1 (psum bank: 512 fp32)
NB = 512                      # mm2 free dim (psum bank: 512 fp32)
KC = 16                       # ko chunks per x DMA (16 KiB/partition)
NCH = KO // KC                # x DMA chunks per token block

# Module-level knobs for test.py (harness never touches these)
TRACE = False
LAST_RESULTS = None

_nc_cache = None


def _build_program():
    from concourse import bacc, mybir, tile

    # Bacc (not raw Bass): its finalize() runs generate_event_semaphores,
    # which splits multi-sem waits to satisfy TRN2's 1-wait-per-instruction
    # hardware constraint (walrus rejects >1 otherwise).
    nc = bacc.Bacc(
        "TRN2", target_bir_lowering=False, debug=False, num_devices=NCORES
    )

    f32 = mybir.dt.float32
    f16 = mybir.dt.float16

    n_blocks = TOK // TB
    xt = nc.dram_tensor(
        "xt", [n_blocks * NCH, P, KC, TB], f16, kind="ExternalInput"
    )
    at = nc.dram_tensor("at", [P, KO, R], f16, kind="ExternalInput")
    bt = nc.dram_tensor("bt", [R, D], f16, kind="ExternalInput")
    y = nc.dram_tensor("y", [TOK, D], f16, kind="ExternalOutput")

    with tile.TileContext(nc) as tc:
        with (
            tc.tile_pool(name="consts", bufs=1) as consts,
            tc.tile_pool(name="xin", bufs=2 * NCH) as xin,
            tc.tile_pool(name="tbuf", bufs=2) as tbuf,
            tc.tile_pool(name="yout", bufs=3) as yout,
            tc.tile_pool(name="pt", bufs=2, space="PSUM") as pt_pool,
            tc.tile_pool(name="py", bufs=3, space="PSUM") as py_pool,
        ):
            at_s = consts.tile([P, KO, R], f16)
            nc.sync.dma_start(at_s[:], at[:])
            bt_s = consts.tile([R, D], f16)
            nc.sync.dma_start(bt_s[:], bt[:])

            # Warm-up matmuls: (a) make PE observe the at/bt DMA sems early,
            # (b) keep PE streaming during the x-DMA prologue so the HAM
            # clock gate reaches K=8/8 before the real matmuls start.
            obs1 = py_pool.tile([R, R], f32, tag="psum_y")
            nc.tensor.matmul(obs1[:], at_s[:, 0, :], at_s[:, 0, :R], start=True, stop=True)
            for _ in range(4):
                warm = py_pool.tile([P, 2, NB], f32, tag="psum_y")
                nc.tensor.matmul(warm[:, 0, :], bt_s[:, :P], bt_s[:, :NB], start=True, stop=True)
            tc.no_sync_barrier()

            def load_x(tb):
                xts = []
                for kc in range(NCH):
                    xt_tile = xin.tile([P, KC, TB], f16, tag="xt")
                    nc.sync.dma_start(xt_tile[:], xt[tb * NCH + kc])
                    xts.append(xt_tile)
                return xts

            def mm1(xts, psum_t):
                for kc in range(NCH):
                    for j in range(KC):
                        ko = kc * KC + j
                        nc.tensor.matmul(
                            psum_t[:],
                            at_s[:, ko, :],
                            xts[kc][:, j, :],
                            start=(ko == 0),
                            stop=(ko == KO - 1),
                        )

            def round_t(psum_t):
                # fp32 PSUM -> fp16 SBUF: the mm2 stationary operand
                tT = tbuf.tile([R, TB], f16)
                nc.vector.tensor_copy(tT[:], psum_t[:])
                return tT

            def mm2_chunk(tb, c, tT):
                y_row = yout.tile([P, D], f16)
                for pair in range(D // (2 * NB)):
                    # Two n-slices into one 2-bank PSUM tile, evacuated by a
                    # single [128, 1024] fp32->fp16 copy (amortizes the
                    # fixed per-op PSUM-read cost).
                    psum_y = py_pool.tile([P, 2, NB], f32, tag="psum_y")
                    for k in range(2):
                        n = 2 * pair + k
                        nc.tensor.matmul(
                            psum_y[:, k, :],
                            tT[:, c * P : (c + 1) * P],
                            bt_s[:, n * NB : (n + 1) * NB],
                            start=True,
                            stop=True,
                        )
                    # Alternate PSUM-evacuation between DVE and ACT so
                    # neither engine gates the tensor engine's psum slots
                    n0 = 2 * pair * NB
                    if pair % 2 == 1:
                        nc.scalar.copy(y_row[:, n0 : n0 + 2 * NB], psum_y[:])
                    else:
                        nc.vector.tensor_copy(y_row[:, n0 : n0 + 2 * NB], psum_y[:])
                row0 = tb * TB + c * P
                # scalar-engine HWDGE ring: offloads the Sync sequencer
                nc.scalar.dma_start(y[row0 : row0 + P, :], y_row[:])

            # PE order must follow x-arrival order (PE is FIFO: a matmul
            # waiting on a late DMA blocks everything behind it).
            for tb in range(n_blocks):
                xts = load_x(tb)
                psum_t = pt_pool.tile([R, TB], f32, tag="psum_t")
                mm1(xts, psum_t)
                tT = round_t(psum_t)
                for c in range(TB // P):
                    mm2_chunk(tb, c, tT)

    nc.finalize()
    return nc


def kernel(x, lora_matrix_B, lora_matrix_A):
    global _nc_cache, LAST_RESULTS
    from concourse.bass_utils import run_bass_kernel_spmd

    if _nc_cache is None:
        _nc_cache = _build_program()
    nc = _nc_cache

    x_flat = np.asarray(x, dtype=np.float32).reshape(NTOK, D).astype(np.float16)
    A = np.asarray(lora_matrix_A, dtype=np.float32).astype(np.float16)
    B = np.asarray(lora_matrix_B, dtype=np.float32).astype(np.float16)

    # at[p, ko, j] = A[j, ko*128 + p];  bt[j, o] = B[o, j]
    at_prep = np.ascontiguousarray(A.reshape(R, KO, P).transpose(2, 1, 0))
    bt_prep = np.ascontiguousarray(B.T)

    n_blocks = TOK // TB
    in_maps = []
    for c in range(NCORES):
        xc = x_flat[c * TOK : (c + 1) * TOK, :]
        # xt[(tb, kc), p, j, t] = xc[tb*TB + t, (kc*KC + j)*128 + p]
        xt_prep = np.ascontiguousarray(
            xc.reshape(n_blocks, TB, NCH, KC, P).transpose(0, 2, 4, 3, 1)
        ).reshape(n_blocks * NCH, P, KC, TB)
        in_maps.append({"xt": xt_prep, "at": at_prep, "bt": bt_prep})

    res = run_bass_kernel_spmd(
        nc, in_maps, core_ids=list(range(NCORES)), trace=TRACE
    )
    LAST_RESULTS = res

    y = np.concatenate(
        [np.asarray(res.results[c]["y"]) for c in range(NCORES)], axis=0
    )
    return y.reshape(BATCH, SEQ, D).astype(np.float32)


# revision 8
# speedup vs baseline: 1.0464x; 1.0464x over previous
"""LoRA linear kernel for Trainium2 (8 NeuronCores, SPMD data-parallel).

Computes y = x @ (B @ A)^T for
    x: [4, 2048, 4096] f32, B: [4096, 16] f32, A: [16, 4096] f32.

Strategy: never materialize W = B @ A.  Factor as t = x @ A^T (rank 16)
then y = t @ B^T.  Tokens (4*2048 = 8192) are sharded across 8 cores
(1024 tokens each); A and B are replicated.

The kernel is HBM-bandwidth bound (~358 GB/s/core), so x is staged and
y is returned in float16 (tolerance is 2e-2; fp16 end-to-end gives
~7e-4), halving HBM traffic vs fp32.  Matmuls run fp16 x fp16 with
fp32 PSUM accumulation; y is cast to fp16 during PSUM evacuation.

DMA-descriptor discipline (the descriptor structure follows the SBUF
tile's innermost contiguous run, so):
  - x is staged chunk-major and loaded into FLAT [128, 8*512] tiles
    -> 8 KiB per-partition descriptors (near line rate).
  - at is staged flat [128, 512] (not [128, 32, 16], which would emit
    4096 32-byte descriptors).
  - y rows are [128, 4096] -> 8 KiB descriptors.

PSUM evacuation: two mm2 n-slices per 2-bank PSUM tile, evacuated by
single [128, 1024] fp32->fp16 copies alternating DVE/ACT.  y DMAs are
issued from the Sync sequencer: an ACT-issued DMA would sit in ACT's
strict-FIFO queue waiting on DVE's copies, stalling ACT's own copies.

Per-core dataflow:
  mm1: t^T[16, tok]  = sum_ko  A^T[ko]  (lhsT [128,16]) . x^T[ko] (rhs [128,512])
  mm2: y[tok128, o]  = t^T[:, chunk] (lhsT [16,128])    . B^T     (rhs [16,512])
  y DMA'd out in natural token-major layout -> host just concatenates.
"""

import sys

import numpy as np

if "/opt/trn_rl_repo" not in sys.path:
    sys.path.insert(0, "/opt/trn_rl_repo")

# Problem shape (hardcoded per contract)
BATCH = 4
SEQ = 2048
D = 4096          # in_features == out_features
R = 16            # lora rank
NCORES = 8
NTOK = BATCH * SEQ            # 8192 tokens total
TOK = NTOK // NCORES          # 1024 tokens per core
P = 128                       # partitions
KO = D // P                   # 32 feature chunks
TB = 512                      # token block for mm1 (psum bank: 512 fp32)
NB = 512                      # mm2 free dim (psum bank: 512 fp32)
KC = 8                        # ko chunks per x DMA (8 KiB/partition)
NCH = KO // KC                # x DMA chunks per token block

# Module-level knobs for test.py (harness never touches these)
TRACE = False
LAST_RESULTS = None

_nc_cache = None


def _build_program():
    from concourse import bacc, mybir, tile

    # Bacc (not raw Bass): its finalize() runs generate_event_semaphores,
    # which splits multi-sem waits to satisfy TRN2's 1-wait-per-instruction
    # hardware constraint (walrus rejects >1 otherwise).
    nc = bacc.Bacc(
        "TRN2", target_bir_lowering=False, debug=False, num_devices=NCORES
    )

    f32 = mybir.dt.float32
    f16 = mybir.dt.float16

    n_blocks = TOK // TB
    xt = nc.dram_tensor(
        "xt", [n_blocks * NCH, P, KC * TB], f16, kind="ExternalInput"
    )
    at = nc.dram_tensor("at", [P, KO * R], f16, kind="ExternalInput")
    bt = nc.dram_tensor("bt", [R, D], f16, kind="ExternalInput")
    y = nc.dram_tensor("y", [TOK, D], f16, kind="ExternalOutput")

    with tile.TileContext(nc) as tc:
        with (
            tc.tile_pool(name="consts", bufs=1) as consts,
            tc.tile_pool(name="xin", bufs=2 * NCH) as xin,
            tc.tile_pool(name="tbuf", bufs=2) as tbuf,
            tc.tile_pool(name="yout", bufs=3) as yout,
            tc.tile_pool(name="pt", bufs=2, space="PSUM") as pt_pool,
            tc.tile_pool(name="py", bufs=3, space="PSUM") as py_pool,
        ):
            at_s = consts.tile([P, KO * R], f16)
            nc.sync.dma_start(at_s[:], at[:])
            bt_s = consts.tile([R, D], f16)
            nc.sync.dma_start(bt_s[:], bt[:])

            # Warm-up matmuls: (a) make PE observe the at/bt DMA sems early,
            # (b) keep PE streaming during the x-DMA prologue so the HAM
            # clock gate reaches K=8/8 before the real matmuls start.
            obs1 = py_pool.tile([R, R], f32, tag="psum_y")
            nc.tensor.matmul(obs1[:], at_s[:, :R], at_s[:, :R], start=True, stop=True)
            for _ in range(4):
                warm = py_pool.tile([P, 2, NB], f32, tag="psum_y")
                nc.tensor.matmul(warm[:, 0, :], bt_s[:, :P], bt_s[:, :NB], start=True, stop=True)
            tc.no_sync_barrier()

            def load_x(tb):
                xts = []
                for kc in range(NCH):
                    xt_tile = xin.tile([P, KC * TB], f16, tag="xt")
                    nc.sync.dma_start(xt_tile[:], xt[tb * NCH + kc])
                    xts.append(xt_tile)
                return xts

            def mm1(xts, psum_t):
                for kc in range(NCH):
                    for j in range(KC):
                        ko = kc * KC + j
                        nc.tensor.matmul(
                            psum_t[:],
                            at_s[:, ko * R : (ko + 1) * R],
                            xts[kc][:, j * TB : (j + 1) * TB],
                            start=(ko == 0),
                            stop=(ko == KO - 1),
                        )

            def round_t(psum_t):
                # fp32 PSUM -> fp16 SBUF: the mm2 stationary operand
                tT = tbuf.tile([R, TB], f16)
                nc.vector.tensor_copy(tT[:], psum_t[:])
                return tT

            def mm2_chunk(tb, c, tT):
                y_row = yout.tile([P, D], f16)
                for pair in range(D // (2 * NB)):
                    # Two n-slices into one 2-bank PSUM tile, evacuated by a
                    # single [128, 1024] fp32->fp16 copy (amortizes the
                    # fixed per-op PSUM-read cost).
                    psum_y = py_pool.tile([P, 2, NB], f32, tag="psum_y")
                    for k in range(2):
                        n = 2 * pair + k
                        nc.tensor.matmul(
                            psum_y[:, k, :],
                            tT[:, c * P : (c + 1) * P],
                            bt_s[:, n * NB : (n + 1) * NB],
                            start=True,
                            stop=True,
                        )
                    # Alternate PSUM-evacuation between DVE and ACT so
                    # neither engine gates the tensor engine's psum slots
                    n0 = 2 * pair * NB
                    if pair % 2 == 1:
                        nc.scalar.copy(y_row[:, n0 : n0 + 2 * NB], psum_y[:])
                    else:
                        nc.vector.tensor_copy(y_row[:, n0 : n0 + 2 * NB], psum_y[:])
                row0 = tb * TB + c * P
                nc.sync.dma_start(y[row0 : row0 + P, :], y_row[:])

            # PE order must follow x-arrival order (PE is FIFO: a matmul
            # waiting on a late DMA blocks everything behind it).
            for tb in range(n_blocks):
                xts = load_x(tb)
                psum_t = pt_pool.tile([R, TB], f32, tag="psum_t")
                mm1(xts, psum_t)
                tT = round_t(psum_t)
                for c in range(TB // P):
                    mm2_chunk(tb, c, tT)

    nc.finalize()
    return nc


def kernel(x, lora_matrix_B, lora_matrix_A):
    global _nc_cache, LAST_RESULTS
    from concourse.bass_utils import run_bass_kernel_spmd

    if _nc_cache is None:
        _nc_cache = _build_program()
    nc = _nc_cache

    x_flat = np.asarray(x, dtype=np.float32).reshape(NTOK, D).astype(np.float16)
    A = np.asarray(lora_matrix_A, dtype=np.float32).astype(np.float16)
    B = np.asarray(lora_matrix_B, dtype=np.float32).astype(np.float16)

    # at[p, ko*R + j] = A[j, ko*128 + p];  bt[j, o] = B[o, j]
    at_prep = np.ascontiguousarray(
        A.reshape(R, KO, P).transpose(2, 1, 0).reshape(P, KO * R)
    )
    bt_prep = np.ascontiguousarray(B.T)

    n_blocks = TOK // TB
    in_maps = []
    for c in range(NCORES):
        xc = x_flat[c * TOK : (c + 1) * TOK, :]
        # xt[(tb, kc), p, j*TB + t] = xc[tb*TB + t, (kc*KC + j)*128 + p]
        xt_prep = np.ascontiguousarray(
            xc.reshape(n_blocks, TB, NCH, KC, P).transpose(0, 2, 4, 3, 1)
        ).reshape(n_blocks * NCH, P, KC * TB)
        in_maps.append({"xt": xt_prep, "at": at_prep, "bt": bt_prep})

    res = run_bass_kernel_spmd(
        nc, in_maps, core_ids=list(range(NCORES)), trace=TRACE
    )
    LAST_RESULTS = res

    y = np.concatenate(
        [np.asarray(res.results[c]["y"]) for c in range(NCORES)], axis=0
    )
    return y.reshape(BATCH, SEQ, D).astype(np.float32)
